# revision 15
# baseline (speedup 1.0000x reference)
"""DetectionLoss Trainium2 Bass kernel.

Strategy (data-parallel over batch, 2 images per core x 8 cores):
  The loss decomposes into per-scale sums that only need:
    - dense: sum_k exp(cls[b,k,cell]) for every cell  (for logsumexp at
      positive cells), and softplus over all obj logits
    - sparse: values at the <=64 box cells per image (obj logit, reg 4-vec,
      cls logit at the argmax-target class, log of the dense sumexp)
  Positive cells, duplicate-cell resolution (last box wins for reg targets,
  lowest label wins for the cls one-hot argmax) and the smooth-L1 are all
  computed on-device with 128x128 compare masks (128 = 2 images x 64 boxes).
  Each core returns 18 partial sums; the host combines them (the global
  npos normalizer makes per-core normalization impossible anyway).
"""

import numpy as np

import concourse.bass as bass
import concourse.tile as tile
from concourse import bacc, mybir
from concourse.bass_utils import run_bass_kernel_spmd
from concourse.tile_rust import add_dep_helper

F32 = mybir.dt.float32
I32 = mybir.dt.int32
AF = mybir.ActivationFunctionType
OP = mybir.AluOpType
AX = mybir.AxisListType

B_TOT = 16
N_CORES = 8
B_SH = B_TOT // N_CORES  # 2 images per core
NBOX = 64
NP = B_SH * NBOX  # 128 partitions: (image, box)
C = 30
SCALES = [(80, 80), (40, 40), (20, 20)]
BIG = 1.0e9
CHUNK = 512

CLS_W, REG_W, OBJ_W = 1.0, 5.0, 1.0

# partials layout: per scale s, cols 6s + [lse, clsval, sl1, obj, softplus, npos]
NPART = 18


def _consts():
    ident = np.eye(128, dtype=np.float32)
    utri = np.triu(np.ones((128, 128), np.float32), 1)
    big = np.concatenate([ident, utri], axis=1)  # [128, 256]

    p = np.arange(128)
    bvec = (p >= NBOX).astype(np.float32)  # image index per partition
    kc = np.zeros((128, 18), np.float32)
    for s, (h, w) in enumerate(SCALES):
        hw = h * w
        for ch in range(4):
            kc[:, 4 * s + ch] = bvec * 4 * hw + ch * hw  # reg gather offsets
        kc[:, 12 + s] = bvec * hw          # key/obj/sumexp offsets
        kc[:, 15 + s] = bvec * C * hw      # cls gather offsets

    # [120, 4]: partition (b, k, half) -> column (b, half)
    bsel = np.zeros((120, 4), np.float32)
    for pp in range(120):
        b = pp // 60
        half = pp % 2
        bsel[pp, b * 2 + half] = 1.0

    ones = np.ones((128, 1), np.float32)
    return big, kc, bsel, ones


def emit(tc: tile.TileContext, outs, ins):
    """outs: partials AP [18]; ins: dict name -> AP (per-core shard shapes)."""
    nc = tc.nc
    out_ap = outs

    big_c, kc_c, bsel_c, ones_c = _consts()
    big_h = nc.inline_tensor(big_c, name="cbig")
    kc_h = nc.inline_tensor(kc_c, name="ckc")
    bsel_h = nc.inline_tensor(bsel_c, name="cbsel")
    ones_h = nc.inline_tensor(ones_c, name="cones")

    pools = []

    def mkpool(**kw):
        p = tc.alloc_tile_pool(**kw)
        pools.append(p)
        return p

    pool = mkpool(name="sb", bufs=1)
    cpool = mkpool(name="chunks", bufs=3)
    mmps = mkpool(name="mmps", bufs=3, space="PSUM")
    tpps = mkpool(name="tpps", bufs=2, space="PSUM")
    lbps = mkpool(name="lbps", bufs=1, space="PSUM")
    fips = mkpool(name="fips", bufs=1, space="PSUM")

    bigt = pool.tile([128, 256], F32, tag="bigt")
    nc.sync.dma_start(out=bigt[:], in_=big_h.ap())
    ident = bigt[:, 0:128]
    utri = bigt[:, 128:256]
    kct = pool.tile([128, 18], F32, tag="kct")
    nc.sync.dma_start(out=kct[:], in_=kc_h.ap())
    bselt = pool.tile([120, 4], F32, tag="bselt")
    nc.sync.dma_start(out=bselt[:], in_=bsel_h.ap())
    onest = pool.tile([128, 1], F32, tag="onest")
    nc.sync.dma_start(out=onest[:], in_=ones_h.ap())

    # ---- boxes / labels ----
    btile = pool.tile([NP, 4], F32, tag="btile")
    nc.sync.dma_start(out=btile[:], in_=ins["boxes"].rearrange("b n c -> (b n) c"))
    labi = pool.tile([NP, 1], I32, tag="labi")
    nc.sync.dma_start(out=labi[:], in_=ins["labels"].rearrange("b n -> (b n)")[:, None])
    labf = pool.tile([NP, 1], F32, tag="labf")
    nc.vector.tensor_copy(out=labf[:], in_=labi[:])

    # label transpose-broadcast: labTm[p, q] = labf[q]
    labTm = lbps.tile([128, 128], F32, tag="labTm")
    nc.tensor.transpose(out=labTm[:], in_=labf[:].to_broadcast([128, 128]), identity=ident)

    # partial-sum stack
    stack = pool.tile([128, NPART], F32, tag="stack")
    nc.vector.memset(stack[:], 0.0)

    # scratch DRAM for per-cell sumexp maps
    se_h = [
        nc.dram_tensor(f"se{s}", (B_SH * h * w,), F32, kind="Internal")
        for s, (h, w) in enumerate(SCALES)
    ]

    for s, (H, W) in enumerate(SCALES):
        HW = H * W
        HW2 = HW // 2
        base = 6 * s

        # ---------- dense: per-cell sum_k exp(cls) -> DRAM ----------
        cls_in = ins[f"cls_p{s}"]
        cls_pf = cls_in.rearrange(
            "b k (u f) w -> (b k u) (f w)", u=2
        )  # [120, HW/2], contiguous rows
        se_flat = se_h[s].ap()
        se_sb = pool.tile([4, HW2], F32, tag=f"sesb{s}")
        nch = (HW2 + CHUNK - 1) // CHUNK
        for ci in range(nch):
            c0 = ci * CHUNK
            n = min(CHUNK, HW2 - c0)
            ct = cpool.tile([120, CHUNK], F32, tag=f"clsin{s}")
            nc.sync.dma_start(out=ct[:, :n], in_=cls_pf[:, c0 : c0 + n])
            et = cpool.tile([120, CHUNK], F32, tag=f"clsexp{s}")
            nc.scalar.activation(out=et[:, :n], in_=ct[:, :n], func=AF.Exp)
            ps = mmps.tile([4, CHUNK], F32, tag="mm")
            nc.tensor.matmul(out=ps[:, :n], lhsT=bselt[:], rhs=et[:, :n], start=True, stop=True)
            # PSUM can't be DMA'd; evacuate via compute engines (alternate to balance)
            if ci % 2 == 0:
                nc.vector.tensor_copy(out=se_sb[:, c0 : c0 + n], in_=ps[:, :n])
            else:
                nc.scalar.copy(out=se_sb[:, c0 : c0 + n], in_=ps[:, :n])
        se_dmas = [
            nc.sync.dma_start(out=se_flat.rearrange("(p f) -> p f", p=4), in_=se_sb[:])
        ]

        # ---------- dense: softplus over all obj logits ----------
        obj_in = ins[f"obj_p{s}"]
        obj_flat = obj_in.rearrange("b c h w -> (b c h w)")
        p_obj = 128 if s < 2 else 32
        f_obj = (B_SH * HW) // p_obj
        objt = pool.tile([p_obj, f_obj], F32, tag=f"objt{s}")
        nc.sync.dma_start(out=objt[:], in_=obj_flat.rearrange("(p f) -> p f", p=p_obj))
        # softplus(x) = ln(exp(x) + 1); gen3 ACT tables lack Softplus but
        # natural_log_exp_and_others has Exp and Ln (bias folds the +1)
        obje = pool.tile([p_obj, f_obj], F32, tag=f"obje{s}")
        nc.scalar.activation(out=obje[:], in_=objt[:], func=AF.Exp)
        objl = pool.tile([p_obj, f_obj], F32, tag=f"objl{s}")
        nc.scalar.activation(
            out=objl[:],
            in_=obje[:],
            func=AF.Ln,
            bias=1.0,
            accum_out=stack[:p_obj, base + 4 : base + 5],
        )

        # ---------- sparse: per-box cells & masks ----------
        # floor(cx*W): HW f32->i32 convert rounds to nearest, so compute
        # cx*W - 0.5 (one dual-op) and let the convert round it.
        gxr = pool.tile([NP, 1], F32, tag=f"gxr{s}")
        nc.vector.tensor_scalar(
            out=gxr[:], in0=btile[:, 0:1], scalar1=float(W), scalar2=-0.5, op0=OP.mult, op1=OP.add
        )
        gyr = pool.tile([NP, 1], F32, tag=f"gyr{s}")
        nc.vector.tensor_scalar(
            out=gyr[:], in0=btile[:, 1:2], scalar1=float(H), scalar2=-0.5, op0=OP.mult, op1=OP.add
        )
        gxi = pool.tile([NP, 1], I32, tag=f"gxi{s}")
        nc.gpsimd.tensor_copy(out=gxi[:], in_=gxr[:])
        gyi = pool.tile([NP, 1], I32, tag=f"gyi{s}")
        nc.gpsimd.tensor_copy(out=gyi[:], in_=gyr[:])
        gxf = pool.tile([NP, 1], F32, tag=f"gxf{s}")
        nc.gpsimd.tensor_copy(out=gxf[:], in_=gxi[:])
        gyf = pool.tile([NP, 1], F32, tag=f"gyf{s}")
        nc.gpsimd.tensor_copy(out=gyf[:], in_=gyi[:])
        nc.vector.tensor_scalar_min(gxf[:], gxf[:], float(W - 1))
        nc.vector.tensor_scalar_min(gyf[:], gyf[:], float(H - 1))

        cellf = pool.tile([NP, 1], F32, tag=f"cellf{s}")
        nc.vector.tensor_scalar(out=cellf[:], in0=gyf[:], scalar1=float(W), scalar2=None, op0=OP.mult)
        nc.vector.tensor_add(cellf[:], cellf[:], gxf[:])

        keyf = pool.tile([NP, 1], F32, tag=f"keyf{s}")
        nc.vector.tensor_add(keyf[:], cellf[:], kct[:, 12 + s : 13 + s])
        keyi = pool.tile([NP, 1], I32, tag=f"keyi{s}")
        nc.gpsimd.tensor_copy(out=keyi[:], in_=keyf[:])

        ridxf = pool.tile([NP, 4], F32, tag=f"ridxf{s}")
        nc.vector.tensor_tensor(
            out=ridxf[:], in0=cellf[:].to_broadcast([NP, 4]), in1=kct[:, 4 * s : 4 * s + 4], op=OP.add
        )
        ridxi = pool.tile([NP, 4], I32, tag=f"ridxi{s}")
        nc.gpsimd.tensor_copy(out=ridxi[:], in_=ridxf[:])

        # key transpose: keyTm[p, q] = keyf[q]
        keyTm = tpps.tile([128, 128], F32, tag="keyTm")
        nc.tensor.transpose(out=keyTm[:], in_=keyf[:].to_broadcast([128, 128]), identity=ident)

        eqm = pool.tile([128, 128], F32, tag=f"eqm{s}")
        nc.vector.tensor_tensor(
            out=eqm[:], in0=keyf[:].to_broadcast([128, 128]), in1=keyTm[:], op=OP.is_equal
        )
        lose = pool.tile([128, 128], F32, tag=f"lose{s}")
        nc.vector.tensor_mul(lose[:], eqm[:], utri)
        losev = pool.tile([NP, 1], F32, tag=f"losev{s}")
        nc.vector.tensor_reduce(out=losev[:], in_=lose[:], axis=AX.X, op=OP.max)
        winner = pool.tile([NP, 1], F32, tag=f"winner{s}")
        nc.vector.tensor_scalar(
            out=winner[:], in0=losev[:], scalar1=-1.0, scalar2=1.0, op0=OP.mult, op1=OP.add
        )
        nc.vector.tensor_copy(out=stack[:, base + 5 : base + 6], in_=winner[:])

        # min same-cell label
        cnd = pool.tile([128, 128], F32, tag=f"cnd{s}")
        nc.vector.tensor_scalar(
            out=cnd[:], in0=eqm[:], scalar1=-BIG, scalar2=BIG, op0=OP.mult, op1=OP.add
        )
        nc.vector.tensor_tensor(out=cnd[:], in0=cnd[:], in1=labTm[:], op=OP.add)
        minlab = pool.tile([NP, 1], F32, tag=f"minlab{s}")
        nc.vector.tensor_reduce(out=minlab[:], in_=cnd[:], axis=AX.X, op=OP.min)

        cidxf = pool.tile([NP, 1], F32, tag=f"cidxf{s}")
        nc.vector.tensor_scalar(out=cidxf[:], in0=minlab[:], scalar1=float(HW), scalar2=None, op0=OP.mult)
        nc.vector.tensor_add(cidxf[:], cidxf[:], cellf[:])
        nc.vector.tensor_add(cidxf[:], cidxf[:], kct[:, 15 + s : 16 + s])
        cidxi = pool.tile([NP, 1], I32, tag=f"cidxi{s}")
        nc.gpsimd.tensor_copy(out=cidxi[:], in_=cidxf[:])

        # ---------- gathers ----------
        objg = pool.tile([NP, 1], F32, tag=f"objg{s}")
        nc.gpsimd.indirect_dma_start(
            out=objg[:],
            out_offset=None,
            in_=obj_flat[:, None],
            in_offset=bass.IndirectOffsetOnAxis(ap=keyi[:, :1], axis=0),
        )
        seg = pool.tile([NP, 1], F32, tag=f"seg{s}")
        g = nc.gpsimd.indirect_dma_start(
            out=seg[:],
            out_offset=None,
            in_=se_flat[:, None],
            in_offset=bass.IndirectOffsetOnAxis(ap=keyi[:, :1], axis=0),
        )
        for d in se_dmas:
            add_dep_helper(g.ins, d.ins, reason="se scratch RAW")
        # one gather per coordinate channel: HW uses one index per partition
        # (a [128,4] offset tile would fetch 4 contiguous elems instead)
        regg = pool.tile([NP, 4], F32, tag=f"regg{s}")
        reg_flat = ins[f"reg_p{s}"].rearrange("b c h w -> (b c h w)")[:, None]
        for ch in range(4):
            nc.gpsimd.indirect_dma_start(
                out=regg[:, ch : ch + 1],
                out_offset=None,
                in_=reg_flat,
                in_offset=bass.IndirectOffsetOnAxis(ap=ridxi[:, ch : ch + 1], axis=0),
            )
        clsvg = pool.tile([NP, 1], F32, tag=f"clsvg{s}")
        nc.gpsimd.indirect_dma_start(
            out=clsvg[:],
            out_offset=None,
            in_=cls_in.rearrange("b k h w -> (b k h w)")[:, None],
            in_offset=bass.IndirectOffsetOnAxis(ap=cidxi[:, :1], axis=0),
        )

        # ---------- losses ----------
        lseg = pool.tile([NP, 1], F32, tag=f"lseg{s}")
        nc.scalar.activation(out=lseg[:], in_=seg[:], func=AF.Ln)
        nc.vector.tensor_mul(stack[:, base + 0 : base + 1], lseg[:], winner[:])
        nc.vector.tensor_mul(stack[:, base + 1 : base + 2], clsvg[:], winner[:])
        nc.vector.tensor_mul(stack[:, base + 3 : base + 4], objg[:], winner[:])

        d4 = pool.tile([NP, 4], F32, tag=f"d4{s}")
        nc.vector.tensor_sub(d4[:], regg[:], btile[:])
        nc.scalar.activation(out=d4[:], in_=d4[:], func=AF.Abs)
        q4 = pool.tile([NP, 4], F32, tag=f"q4{s}")
        nc.vector.tensor_scalar_min(q4[:], d4[:], 1.0)
        hq = pool.tile([NP, 4], F32, tag=f"hq{s}")
        nc.vector.tensor_scalar(out=hq[:], in0=q4[:], scalar1=-0.5, scalar2=None, op0=OP.mult)
        nc.vector.tensor_add(hq[:], hq[:], d4[:])
        nc.vector.tensor_mul(hq[:], hq[:], q4[:])
        sl1 = pool.tile([NP, 1], F32, tag=f"sl1{s}")
        nc.vector.tensor_reduce(out=sl1[:], in_=hq[:], axis=AX.X, op=OP.add)
        nc.vector.tensor_scalar(out=sl1[:], in0=sl1[:], scalar1=0.25, scalar2=None, op0=OP.mult)
        nc.vector.tensor_scalar_min(sl1[:], sl1[:], 10.0)
        nc.vector.tensor_mul(stack[:, base + 2 : base + 3], sl1[:], winner[:])

    # ---------- final reduction: [18] = stack^T @ ones ----------
    fin = fips.tile([NPART, 1], F32, tag="fin")
    nc.tensor.matmul(out=fin[:], lhsT=stack[:], rhs=onest[:], start=True, stop=True)
    fin_sb = pool.tile([NPART, 1], F32, tag="fin_sb")
    nc.vector.tensor_copy(out=fin_sb[:], in_=fin[:])
    nc.sync.dma_start(out=out_ap, in_=fin_sb[:])

    for p in reversed(pools):
        p.release()


# ---------------------------------------------------------------------------
# host side
# ---------------------------------------------------------------------------

_CACHE = {}


def _build():
    if "nc" in _CACHE:
        return _CACHE["nc"]
    nc = bacc.Bacc(
        "TRN2",
        target_bir_lowering=False,
        debug=False,
        enable_asserts=False,
        num_devices=N_CORES,
    )
    ins = {}
    for s, (h, w) in enumerate(SCALES):
        ins[f"cls_p{s}"] = nc.dram_tensor(f"cls_p{s}", (B_SH, C, h, w), F32, kind="ExternalInput").ap()
        ins[f"reg_p{s}"] = nc.dram_tensor(f"reg_p{s}", (B_SH, 4, h, w), F32, kind="ExternalInput").ap()
        ins[f"obj_p{s}"] = nc.dram_tensor(f"obj_p{s}", (B_SH, 1, h, w), F32, kind="ExternalInput").ap()
    ins["boxes"] = nc.dram_tensor("boxes", (B_SH, NBOX, 4), F32, kind="ExternalInput").ap()
    ins["labels"] = nc.dram_tensor("labels", (B_SH, NBOX), I32, kind="ExternalInput").ap()
    out = nc.dram_tensor("partials", (NPART,), F32, kind="ExternalOutput").ap()

    with tile.TileContext(nc) as tc:
        emit(tc, out, ins)
    nc.compile()
    _CACHE["nc"] = nc
    return nc


def combine_partials(parts):
    """parts: [n_cores, 18] -> final [4] losses."""
    tot = np.asarray(parts, np.float64).sum(axis=0)
    cls_sum = reg_sum = obj_sum = 0.0
    for s, (h, w) in enumerate(SCALES):
        b = 6 * s
        lse, val, sl1, obj, sp, npos = tot[b : b + 6]
        npos = max(npos, 1.0)
        cls_sum += (lse - val) / npos * CLS_W
        reg_sum += sl1 / npos * REG_W
        obj_sum += (sp - obj) / (B_TOT * h * w) * OBJ_W
    cls_sum /= len(SCALES)
    reg_sum /= len(SCALES)
    obj_sum /= len(SCALES)
    total = cls_sum + reg_sum + obj_sum
    return np.array([total, cls_sum, reg_sum, obj_sum], np.float32)


TRACE = False
LAST_RESULT = None


def kernel(**inputs):
    global LAST_RESULT
    nc = _build()
    in_maps = []
    for c in range(N_CORES):
        lo, hi = c * B_SH, (c + 1) * B_SH
        m = {}
        for s in range(3):
            m[f"cls_p{s}"] = np.ascontiguousarray(inputs[f"cls_p{s}"][lo:hi])
            m[f"reg_p{s}"] = np.ascontiguousarray(inputs[f"reg_p{s}"][lo:hi])
            m[f"obj_p{s}"] = np.ascontiguousarray(inputs[f"obj_p{s}"][lo:hi])
        m["boxes"] = np.ascontiguousarray(inputs["boxes"][lo:hi])
        m["labels"] = np.ascontiguousarray(inputs["labels"][lo:hi])
        in_maps.append(m)
    res = run_bass_kernel_spmd(
        nc, in_maps, core_ids=list(range(N_CORES)), trace=TRACE
    )
    LAST_RESULT = res
    parts = np.stack([np.asarray(r["partials"]) for r in res.results])
    return combine_partials(parts)


# revision 26
# speedup vs baseline: 1.1496x; 1.1496x over previous
"""DetectionLoss Trainium2 Bass kernel.

Data-parallel over batch: 2 images per core x 8 cores; host sums 18 partial
sums per core (npos is a global normalizer, so per-core normalization is
impossible anyway - the sharding hint's "per-shard sums + counts").

Device algorithm per core:
  sparse path (starts immediately): box cells -> 128x128 same-cell masks
  (last-box-wins winners, min-label targets) -> indirect gathers of the
  per-cell records (obj, reg0..3) and cls logit at the target class ->
  smooth-L1 and CE numerators.
  dense path (overlapped): sum_k exp(cls[k, cell]) for every cell via
  bf16 matmul against a block-selector, staged to DRAM, gathered back at
  the <=128 positive cells for the logsumexp term; softplus over all obj
  logits via Exp+Ln(x+1) (gen3 ACT tables lack Softplus).

The obj+reg inputs are repacked on host into per-cell records [2HW, 5]
(pure relayout - all arithmetic happens on device) so one indirect DMA per
scale fetches all five values per box; indirect DMAs cost ~1.1us each on
GPSIMD and were the dominant serial chain in v1.
"""

import numpy as np
import ml_dtypes

import concourse.bass as bass
import concourse.tile as tile
from concourse import bacc, mybir
from concourse.bass_utils import run_bass_kernel_spmd
from concourse.tile_rust import add_dep_helper

F32 = mybir.dt.float32
BF16 = mybir.dt.bfloat16
I32 = mybir.dt.int32
AF = mybir.ActivationFunctionType
OP = mybir.AluOpType
AX = mybir.AxisListType

B_TOT = 16
N_CORES = 8
B_SH = B_TOT // N_CORES
NBOX = 64
NP = B_SH * NBOX  # 128 partitions: (image, box)
C = 30
SCALES = [(80, 80), (40, 40), (20, 20)]
BIG = 1.0e9
CHUNK = 400  # divides every HW/2; psum [4*nch, 400] fits one bank

CLS_W, REG_W, OBJ_W = 1.0, 5.0, 1.0
NPART = 18  # per scale s, cols 6s + [lse, clsval, sl1, obj, softplus, npos]


def _consts():
    ident = np.eye(128, dtype=np.float32)
    utri = np.triu(np.ones((128, 128), np.float32), 1)
    big = np.concatenate([ident, utri], axis=1)  # [128, 256]

    p = np.arange(128)
    bvec = (p >= NBOX).astype(np.float32)
    kc = np.zeros((128, 24), np.float32)
    for s, (h, w) in enumerate(SCALES):
        hw = h * w
        kc[:, 0 + s] = w          # W
        kc[:, 3 + s] = h          # H
        kc[:, 6 + s] = w - 1
        kc[:, 9 + s] = h - 1
        kc[:, 12 + s] = bvec * hw          # key offset
        kc[:, 15 + s] = bvec * C * hw      # cls gather offset
        kc[:, 18 + s] = hw                 # for minlab*HW

    # [120, 4]: partition (b, k, u) -> column (b*2 + u)
    bsel = np.zeros((120, 4), ml_dtypes.bfloat16)
    for pp in range(120):
        b = pp // 60
        u = pp % 2
        bsel[pp, b * 2 + u] = 1.0

    ones = np.ones((128, 1), np.float32)
    return big, kc, bsel, ones


def emit(tc: tile.TileContext, outs, ins):
    """outs: partials AP [18]; ins: dict name -> AP (per-core shard shapes)."""
    nc = tc.nc
    out_ap = outs

    big_c, kc_c, bsel_c, ones_c = _consts()
    big_h = nc.inline_tensor(big_c, name="cbig")
    kc_h = nc.inline_tensor(kc_c, name="ckc")
    bsel_h = nc.inline_tensor(bsel_c, name="cbsel")
    ones_h = nc.inline_tensor(ones_c, name="cones")

    pools = []

    def mkpool(**kw):
        p = tc.alloc_tile_pool(**kw)
        pools.append(p)
        return p

    pool = mkpool(name="sb", bufs=1)
    seps = mkpool(name="seps", bufs=3, space="PSUM")
    kmps = mkpool(name="kmps", bufs=2, space="PSUM")
    lbps = mkpool(name="lbps", bufs=1, space="PSUM")
    fips = mkpool(name="fips", bufs=1, space="PSUM")

    # ---- tiny inputs first: the sparse chain is the critical path ----
    btile = pool.tile([NP, 4], F32, tag="btile")
    nc.sync.dma_start(out=btile[:], in_=ins["boxes"].rearrange("b n c -> (b n) c"))
    labi = pool.tile([NP, 1], I32, tag="labi")
    nc.sync.dma_start(out=labi[:], in_=ins["labels"].rearrange("b n -> (b n)")[:, None])

    bigt = pool.tile([128, 256], F32, tag="bigt")
    nc.sync.dma_start(out=bigt[:], in_=big_h.ap())
    utri = bigt[:, 128:256]
    kct = pool.tile([128, 24], F32, tag="kct")
    nc.sync.dma_start(out=kct[:], in_=kc_h.ap())
    bselt = pool.tile([120, 4], BF16, tag="bselt")
    nc.sync.dma_start(out=bselt[:], in_=bsel_h.ap())

    labf = pool.tile([NP, 1], F32, tag="labf")
    nc.vector.tensor_copy(out=labf[:], in_=labi[:])

    # ---- batched (all scales) box -> cell/key indices; [128, 3] ops ----
    # floor(x) = round-to-nearest(x - 0.5): HW f32->i32 convert rounds
    gxr = pool.tile([NP, 3], F32, tag="gxr")
    nc.vector.tensor_tensor(out=gxr[:], in0=btile[:, 0:1].to_broadcast([NP, 3]), in1=kct[:, 0:3], op=OP.mult)
    nc.vector.tensor_scalar(out=gxr[:], in0=gxr[:], scalar1=-0.5, scalar2=None, op0=OP.add)
    gyr = pool.tile([NP, 3], F32, tag="gyr")
    nc.vector.tensor_tensor(out=gyr[:], in0=btile[:, 1:2].to_broadcast([NP, 3]), in1=kct[:, 3:6], op=OP.mult)
    nc.vector.tensor_scalar(out=gyr[:], in0=gyr[:], scalar1=-0.5, scalar2=None, op0=OP.add)
    gxi = pool.tile([NP, 3], I32, tag="gxi")
    nc.vector.tensor_copy(out=gxi[:], in_=gxr[:])
    gyi = pool.tile([NP, 3], I32, tag="gyi")
    nc.vector.tensor_copy(out=gyi[:], in_=gyr[:])
    gxf = pool.tile([NP, 3], F32, tag="gxf")
    nc.vector.tensor_copy(out=gxf[:], in_=gxi[:])
    gyf = pool.tile([NP, 3], F32, tag="gyf")
    nc.vector.tensor_copy(out=gyf[:], in_=gyi[:])
    nc.vector.tensor_tensor(out=gxf[:], in0=gxf[:], in1=kct[:, 6:9], op=OP.min)
    nc.vector.tensor_tensor(out=gyf[:], in0=gyf[:], in1=kct[:, 9:12], op=OP.min)

    cellf = pool.tile([NP, 3], F32, tag="cellf")
    nc.vector.tensor_tensor(out=cellf[:], in0=gyf[:], in1=kct[:, 0:3], op=OP.mult)
    nc.vector.tensor_add(cellf[:], cellf[:], gxf[:])
    keyf = pool.tile([NP, 3], F32, tag="keyf")
    nc.vector.tensor_add(keyf[:], cellf[:], kct[:, 12:15])
    keyi = pool.tile([NP, 3], I32, tag="keyi")
    nc.vector.tensor_copy(out=keyi[:], in_=keyf[:])

    # ---- key/label row matrices: PE transpose of broadcast columns ----
    # (labmat[p, q] = labf[q]; keymat_s[p, q] = keyf[q, s])
    labmat = lbps.tile([128, 128], F32, tag="labmat")
    nc.tensor.transpose(
        out=labmat[:], in_=labf[:].to_broadcast([128, 128]), identity=bigt[:, 0:128]
    )

    # ---- per-scale masks: winners (last box wins) + min same-cell label ----
    win3 = pool.tile([NP, 3], F32, tag="win3")
    minlab3 = pool.tile([NP, 3], F32, tag="minlab3")
    for s in range(3):
        kmat = kmps.tile([128, 128], F32, tag="kmat")
        nc.tensor.transpose(
            out=kmat[:],
            in_=keyf[:, s : s + 1].to_broadcast([128, 128]),
            identity=bigt[:, 0:128],
        )
        eqm = pool.tile([128, 128], F32, tag=f"eqm{s}")
        nc.vector.tensor_scalar(
            out=eqm[:], in0=kmat[:], scalar1=keyf[:, s : s + 1], scalar2=None, op0=OP.is_equal
        )
        lose = pool.tile([128, 128], F32, tag=f"lose{s}")
        nc.vector.tensor_mul(lose[:], eqm[:], utri)
        losev = pool.tile([NP, 1], F32, tag=f"losev{s}")
        nc.vector.tensor_reduce(out=losev[:], in_=lose[:], axis=AX.X, op=OP.max)
        nc.vector.tensor_scalar(
            out=win3[:, s : s + 1], in0=losev[:], scalar1=-1.0, scalar2=1.0, op0=OP.mult, op1=OP.add
        )
        cnd = pool.tile([128, 128], F32, tag=f"cnd{s}")
        nc.vector.tensor_scalar(
            out=cnd[:], in0=eqm[:], scalar1=-BIG, scalar2=BIG, op0=OP.mult, op1=OP.add
        )
        nc.vector.tensor_tensor(out=cnd[:], in0=cnd[:], in1=labmat[:], op=OP.add)
        nc.vector.tensor_reduce(out=minlab3[:, s : s + 1], in_=cnd[:], axis=AX.X, op=OP.min)

    cidxf = pool.tile([NP, 3], F32, tag="cidxf")
    nc.vector.tensor_tensor(out=cidxf[:], in0=minlab3[:], in1=kct[:, 18:21], op=OP.mult)
    nc.vector.tensor_add(cidxf[:], cidxf[:], cellf[:])
    nc.vector.tensor_add(cidxf[:], cidxf[:], kct[:, 15:18])
    cidxi = pool.tile([NP, 3], I32, tag="cidxi")
    nc.vector.tensor_copy(out=cidxi[:], in_=cidxf[:])

    # ---- sparse gathers (GPSIMD ~1.1us each: keep the count minimal) ----
    og_all = pool.tile([NP, 15], F32, tag="og_all")  # (obj, reg0..3) x 3 scales
    clsv3 = pool.tile([NP, 3], F32, tag="clsv3")
    for s in range(3):
        nc.gpsimd.indirect_dma_start(
            out=og_all[:, 5 * s : 5 * s + 5],
            out_offset=None,
            in_=ins[f"objreg{s}"],
            in_offset=bass.IndirectOffsetOnAxis(ap=keyi[:, s : s + 1], axis=0),
        )
        nc.gpsimd.indirect_dma_start(
            out=clsv3[:, s : s + 1],
            out_offset=None,
            in_=ins[f"cls_p{s}"].rearrange("b k h w -> (b k h w)")[:, None],
            in_offset=bass.IndirectOffsetOnAxis(ap=cidxi[:, s : s + 1], axis=0),
        )

    # ---- smooth-L1 over gathered reg records, batched [128, 3, 4] ----
    ogv = og_all[:].rearrange("p (s r) -> p s r", r=5)
    d12 = pool.tile([NP, 3, 4], F32, tag="d12")
    nc.vector.tensor_tensor(
        out=d12[:], in0=ogv[:, :, 1:5], in1=btile[:, None, :].to_broadcast([NP, 3, 4]), op=OP.subtract
    )
    nc.scalar.activation(out=d12[:], in_=d12[:], func=AF.Abs)
    q12 = pool.tile([NP, 3, 4], F32, tag="q12")
    nc.vector.tensor_scalar_min(q12[:], d12[:], 1.0)
    h12 = pool.tile([NP, 3, 4], F32, tag="h12")
    nc.vector.tensor_scalar(out=h12[:], in0=q12[:], scalar1=-0.5, scalar2=None, op0=OP.mult)
    nc.vector.tensor_add(h12[:], h12[:], d12[:])
    nc.vector.tensor_mul(h12[:], h12[:], q12[:])
    sl13 = pool.tile([NP, 3], F32, tag="sl13")
    nc.vector.tensor_reduce(out=sl13[:], in_=h12[:], axis=AX.X, op=OP.add)
    nc.vector.tensor_scalar(out=sl13[:], in0=sl13[:], scalar1=0.25, scalar2=None, op0=OP.mult)
    nc.vector.tensor_scalar_min(sl13[:], sl13[:], 10.0)

    # ---- partials stack [128, 18]; cols 6s+j written as strided [128,3] ----
    stack = pool.tile([128, NPART], F32, tag="stack")
    nc.vector.memset(stack[:], 0.0)
    stv = stack[:].rearrange("p (s j) -> p s j", j=6)
    nc.vector.tensor_mul(stv[:, :, 1], clsv3[:], win3[:])
    nc.vector.tensor_mul(stv[:, :, 2], sl13[:], win3[:])
    nc.vector.tensor_mul(stv[:, :, 3], ogv[:, :, 0], win3[:])
    nc.vector.tensor_copy(out=stv[:, :, 5], in_=win3[:])

    # ---- dense: softplus over all obj logits (strided col 0 of records) ----
    for s, (H, W) in enumerate(SCALES):
        HW = H * W
        p_obj = 128 if s < 2 else 32
        n_rec = B_SH * HW // p_obj
        objt = pool.tile([p_obj, n_rec * 5], F32, tag=f"objt{s}")
        nc.sync.dma_start(
            out=objt[:], in_=ins[f"objreg{s}"].rearrange("v r -> (v r)").rearrange("(p f) -> p f", p=p_obj)
        )
        objv = objt[:].rearrange("p (j r) -> p j r", r=5)[:, :, 0]
        obje = pool.tile([p_obj, n_rec], F32, tag=f"obje{s}")
        nc.scalar.activation(out=obje[:], in_=objv, func=AF.Exp)
        objl = pool.tile([p_obj, n_rec], F32, tag=f"objl{s}")
        nc.scalar.activation(
            out=objl[:], in_=obje[:], func=AF.Ln, bias=1.0,
            accum_out=stack[:p_obj, 6 * s + 4 : 6 * s + 5],
        )

    # ---- dense: per-cell sumexp of cls logits -> DRAM -> gather at boxes ----
    se_h = [
        nc.dram_tensor(f"se{s}", (B_SH * h * w,), F32, kind="Internal")
        for s, (h, w) in enumerate(SCALES)
    ]
    seg3 = pool.tile([NP, 3], F32, tag="seg3")
    for s, (H, W) in enumerate(SCALES):
        HW = H * W
        HW2 = HW // 2
        nch = HW2 // CHUNK if HW2 >= CHUNK else 1
        csz = HW2 // nch  # 400, 400, 200
        cls_pf = ins[f"cls_p{s}"].rearrange("b k (u f) w -> (b k u) (f w)", u=2)

        expt = pool.tile([120, HW2], BF16, tag=f"expt{s}")
        ndma = 2 if s == 0 else 1
        dsz = HW2 // ndma
        for di in range(ndma):
            ct = pool.tile([120, dsz], F32, tag=f"clsin{s}_{di}")
            nc.sync.dma_start(out=ct[:], in_=cls_pf[:, di * dsz : (di + 1) * dsz])
            nc.scalar.activation(out=expt[:, di * dsz : (di + 1) * dsz], in_=ct[:], func=AF.Exp)

        # matmul PSUM outputs must start at partition 0/32/64/96: park each
        sesb = pool.tile([4, HW2], F32, tag=f"sesb{s}")
        for ci in range(nch):
            se_ps = seps.tile([4, csz], F32, tag="seps")
            nc.tensor.matmul(
                out=se_ps[:],
                lhsT=bselt[:],
                rhs=expt[:, ci * csz : (ci + 1) * csz],
                start=True,
                stop=True,
            )
            dst = sesb[:, ci * csz : (ci + 1) * csz]
            if ci % 2 == 0:
                nc.vector.tensor_copy(out=dst, in_=se_ps[:])
            else:
                nc.scalar.copy(out=dst, in_=se_ps[:])
        # se flat layout is (b, u, j) = row-major [4, HW2]
        d = nc.sync.dma_start(
            out=se_h[s].ap().rearrange("(p f) -> p f", p=4), in_=sesb[:]
        )
        g = nc.gpsimd.indirect_dma_start(
            out=seg3[:, s : s + 1],
            out_offset=None,
            in_=se_h[s].ap()[:, None],
            in_offset=bass.IndirectOffsetOnAxis(ap=keyi[:, s : s + 1], axis=0),
        )
        add_dep_helper(g.ins, d.ins, reason="se scratch RAW")

    lse3 = pool.tile([NP, 3], F32, tag="lse3")
    nc.scalar.activation(out=lse3[:], in_=seg3[:], func=AF.Ln)
    nc.vector.tensor_mul(stv[:, :, 0], lse3[:], win3[:])

    # ---- final: transpose stack then sum along free (the v1 stack@ones
    # matmul showed a pathological 12us slice) ----
    finT = fips.tile([NPART, 128], F32, tag="finT")
    nc.tensor.transpose(out=finT[:], in_=stack[:], identity=bigt[:, 0:128])
    fin_sb = pool.tile([NPART, 1], F32, tag="fin_sb")
    nc.vector.tensor_reduce(out=fin_sb[:], in_=finT[:], axis=AX.X, op=OP.add)
    nc.sync.dma_start(out=out_ap, in_=fin_sb[:])

    for p in reversed(pools):
        p.release()


# ---------------------------------------------------------------------------
# host side
# ---------------------------------------------------------------------------

_CACHE = {}


def _build():
    if "nc" in _CACHE:
        return _CACHE["nc"]
    nc = bacc.Bacc(
        "TRN2",
        target_bir_lowering=False,
        debug=False,
        enable_asserts=False,
        num_devices=N_CORES,
    )
    ins = {}
    for s, (h, w) in enumerate(SCALES):
        ins[f"cls_p{s}"] = nc.dram_tensor(f"cls_p{s}", (B_SH, C, h, w), F32, kind="ExternalInput").ap()
        ins[f"objreg{s}"] = nc.dram_tensor(f"objreg{s}", (B_SH * h * w, 5), F32, kind="ExternalInput").ap()
    ins["boxes"] = nc.dram_tensor("boxes", (B_SH, NBOX, 4), F32, kind="ExternalInput").ap()
    ins["labels"] = nc.dram_tensor("labels", (B_SH, NBOX), I32, kind="ExternalInput").ap()
    out = nc.dram_tensor("partials", (NPART,), F32, kind="ExternalOutput").ap()

    with tile.TileContext(nc) as tc:
        emit(tc, out, ins)
    nc.compile()
    _CACHE["nc"] = nc
    return nc


def make_objreg(obj_slice, reg_slice):
    """[b,1,H,W] obj + [b,4,H,W] reg -> per-cell records [b*H*W, 5]."""
    b = obj_slice.shape[0]
    hw = obj_slice.shape[2] * obj_slice.shape[3]
    rec = np.empty((b * hw, 5), np.float32)
    rec[:, 0] = np.asarray(obj_slice).reshape(-1)
    rec[:, 1:] = np.asarray(reg_slice).reshape(b, 4, hw).transpose(0, 2, 1).reshape(b * hw, 4)
    return rec


def combine_partials(parts):
    """parts: [n_cores, 18] -> final [4] losses."""
    tot = np.asarray(parts, np.float64).sum(axis=0)
    cls_sum = reg_sum = obj_sum = 0.0
    for s, (h, w) in enumerate(SCALES):
        b = 6 * s
        lse, val, sl1, obj, sp, npos = tot[b : b + 6]
        npos = max(npos, 1.0)
        cls_sum += (lse - val) / npos * CLS_W
        reg_sum += sl1 / npos * REG_W
        obj_sum += (sp - obj) / (B_TOT * h * w) * OBJ_W
    cls_sum /= len(SCALES)
    reg_sum /= len(SCALES)
    obj_sum /= len(SCALES)
    total = cls_sum + reg_sum + obj_sum
    return np.array([total, cls_sum, reg_sum, obj_sum], np.float32)


TRACE = False
LAST_RESULT = None


def kernel(**inputs):
    global LAST_RESULT
    nc = _build()
    in_maps = []
    for c in range(N_CORES):
        lo, hi = c * B_SH, (c + 1) * B_SH
        m = {}
        for s in range(3):
            m[f"cls_p{s}"] = np.ascontiguousarray(inputs[f"cls_p{s}"][lo:hi])
            m[f"objreg{s}"] = make_objreg(
                inputs[f"obj_p{s}"][lo:hi], inputs[f"reg_p{s}"][lo:hi]
            )
        m["boxes"] = np.ascontiguousarray(inputs["boxes"][lo:hi])
        m["labels"] = np.ascontiguousarray(inputs["labels"][lo:hi])
        in_maps.append(m)
    res = run_bass_kernel_spmd(
        nc, in_maps, core_ids=list(range(N_CORES)), trace=TRACE
    )
    LAST_RESULT = res
    parts = np.stack([np.asarray(r["partials"]) for r in res.results])
    return combine_partials(parts)
